# revision 64
# baseline (speedup 1.0000x reference)
"""Routed low-rank FFN (MoE-style) Trainium2 kernel.

out[n] = x[n] @ U[pids[n]] @ V[pids[n]] + bias

Strategy (expert-parallel over 8 NeuronCores), ~30us HW vs 50us baseline:
  - Host: stable-sort tokens by pid; expert p's tokens go to core p // 8.
    Each expert's token list is split into chunks of <= 128 tokens
    ("groups"); every core runs the same static program over G groups of
    capacity C (zero-padded), so the SPMD program is identical on all
    cores while the data differs.
  - Reduced-precision I/O (the kernel is DMA-bound at f32; the harness
    gate is rel_err < 2e-2, measured ~9.2e-3 for this mix):
      x, U, V, bias -> fp8 e4m3; h, out -> fp16; PSUM accumulates f32.
    The fp8 bias row's quantization error is corrected EXACTLY on the
    host by adding (bias - fp8(bias)) to the unpacked output.
  - Device, per group g (one expert's <=C tokens):
      h^T [64, C]    = sum_k U_chunk[k].T @ x_chunk[k]  (fp8, K=128,
        column-split into two 32-wide PE tiles so each LDWEIGHTS hides
        under the other tile's matmul; PSUM pre-zeroed by DVE so all
        matmuls run start=False and the tiles share one bank)
      out [C, 1024]  = [h^T; ones].T @ [V; bias]  (2 matmuls, fp16 x fp8,
        N=512 each into separate single-bank PSUM tiles)
  - Software pipeline at depth 2 (mm1(g) issues before mm2(g-2)) keeps
    the PE stream gapless; a 6-matmul warmup burst fills the dead head
    window and one small dummy matmul per group keeps PE duty high so
    the HAM clock un-throttle (1.2 -> 2.4 GHz) engages and sticks.
  - DMA: all loads on the sync HWDGE ring, interleaved in consumption
    order (x+U slice, V slice, ...; never SWDGE - its Q7 descriptor
    generation costs ~2us per DMA). Epilogue PSUM->SBUF casts split
    across ScalarE/VectorE halves; fp16 stores go out in pairs of
    groups alternating the two HWDGE rings, with the final pair split
    across both rings to shorten the last drain.
  - Host: inverse-permute rows back to original token order, cast f32.
"""

import os

import numpy as np
import ml_dtypes

N_CORES = 8
D_IN = 1024
RANK = 64
D_OUT = 1024
KC = 8  # number of 128-deep contraction chunks: D_IN // 128
MAX_CHUNK = 128  # max tokens per group (PE partition limit for matmul 2)

F8 = ml_dtypes.float8_e4m3
F16 = np.float16

# Set by kernel() after a traced run (KERNEL_TRACE=1): HW kernel span in ns.
LAST_EXEC_TIME_NS = None
LAST_RESULTS = None

_PROGRAM_CACHE = {}


def _slice_bounds(G, n_slices):
    """First slice small so compute starts early, rest even."""
    n_slices = max(1, min(n_slices, G))
    if n_slices == 1:
        return [0, G]
    rest = G - 1
    k = n_slices - 1
    bounds = [0, 1]
    for i in range(1, k + 1):
        bounds.append(1 + round(i * rest / k))
    return bounds


def _build_program(G: int, cvec: tuple):
    """Build the SPMD Bass/Tile program: G groups per core; group g has
    token capacity cvec[g] (non-increasing)."""
    import concourse.tile as tile
    from concourse import bacc, mybir

    nc = bacc.Bacc(
        "TRN2",
        target_bir_lowering=False,
        debug=False,
        enable_asserts=False,
        num_devices=N_CORES,
    )
    f32 = mybir.dt.float32
    f16 = mybir.dt.float16
    f8 = mybir.dt.float8e4

    C0 = cvec[0]
    # flat per-(partition, group) row: KC x-chunks of C_g, then KC u-chunks
    fws = [KC * c + KC * RANK for c in cvec]
    xoffs = [sum(fws[:g]) for g in range(G + 1)]
    FWT = xoffs[G]

    Gp = (G + 1) // 2  # output stores go out in pairs of groups
    xu_d = nc.dram_tensor("xug", [128, FWT], f8, kind="ExternalInput")
    vb_d = nc.dram_tensor("vbg", [RANK + 1, G, D_OUT], f8, kind="ExternalInput")
    o_d = nc.dram_tensor("og", [Gp, C0, 2, D_OUT], f16, kind="ExternalOutput")

    n2 = D_OUT // 512  # matmul-2 free-dim splits (one PSUM bank each)

    xbounds = _slice_bounds(G, 3)  # xu slices
    vbounds = _slice_bounds(G, 3)  # vb slices

    with tile.TileContext(nc) as tc:
        with (
            tc.tile_pool(name="xin", bufs=1) as xpool,
            tc.tile_pool(name="win", bufs=1) as wpool,
            tc.tile_pool(name="hbuf", bufs=1) as hpool,
            tc.tile_pool(name="obuf", bufs=3) as opool,
            tc.tile_pool(name="ph", bufs=1, space="PSUM") as phpool,
            tc.tile_pool(name="po", bufs=1, space="PSUM") as popool,
            tc.tile_pool(name="wm", bufs=1, space="PSUM") as wmpool,
        ):
            # f32 ones row; the [hT; ones] trick folds the bias matmul row.
            ones_sb = wpool.tile([1, C0], f32, tag="ones")
            nc.vector.memset(ones_sb[:], 1.0)

            # Three hT buffers with the fp16 ones row prefilled once.
            hTs = [
                hpool.tile([RANK + 1, C0], f16, tag=f"h{i}", name=f"hT{i}")
                for i in range(3)
            ]
            for i in range(3):
                nc.vector.tensor_copy(hTs[i][RANK : RANK + 1, :], ones_sb[:])

            # HAM warmup: dense N=512 matmuls sized to fit inside the dead
            # head window (preamble + first DMA latency), priming the PE
            # clock un-throttle at zero critical-path cost.
            wm_lhs = wpool.tile([RANK + 1, 32], f16, tag="wml")
            wm_rhs = wpool.tile([RANK + 1, 512], f16, tag="wmr")
            nc.vector.memset(wm_lhs[:], 0.0)
            nc.vector.memset(wm_rhs[:], 0.0)
            wm_ps = wmpool.tile([32, 512], f32, tag="wm")
            for _ in range(6):
                nc.tensor.matmul(
                    wm_ps[:], lhsT=wm_lhs[:], rhs=wm_rhs[:], start=True, stop=True
                )

            def keep_warm(g):
                # one modest dummy matmul per group keeps PE activity dense
                # enough for the HAM clock un-throttle to engage and stick.
                nc.tensor.matmul(
                    wm_ps[:, 0:256],
                    lhsT=wm_lhs[:],
                    rhs=wm_rhs[:, 0:256],
                    start=True,
                    stop=True,
                )

            # All loads on ONE HWDGE ring (sync), interleaved in the order
            # compute consumes them: the ring drains FIFO at the full
            # engine-aggregate rate, so slice k never waits behind bytes
            # that are only needed later. Stores go on the other ring.
            xu_parts, vb_parts = [], []
            for s in range(max(len(xbounds), len(vbounds)) - 1):
                if s < len(xbounds) - 1:
                    g0, g1 = xbounds[s], xbounds[s + 1]
                    w0, w1 = xoffs[g0], xoffs[g1]
                    xu_sb = xpool.tile([128, w1 - w0], f8, tag=f"xu{s}")
                    nc.sync.dma_start(out=xu_sb[:], in_=xu_d[:, w0:w1])
                    xu_parts.append(xu_sb)
                if s < len(vbounds) - 1:
                    g0, g1 = vbounds[s], vbounds[s + 1]
                    vb_sb = wpool.tile([RANK + 1, g1 - g0, D_OUT], f8, tag=f"vb{s}")
                    nc.sync.dma_start(out=vb_sb[:], in_=vb_d[:, g0:g1])
                    vb_parts.append(vb_sb)

            def slice_of(g):
                sx = next(i for i in range(len(xbounds) - 1) if xbounds[i + 1] > g)
                sv = next(i for i in range(len(vbounds) - 1) if vbounds[i + 1] > g)
                return (
                    xu_parts[sx],
                    xoffs[g] - xoffs[xbounds[sx]],
                    vb_parts[sv],
                    g - vbounds[sv],
                )

            # Software pipeline over groups: issue mm1(g) before mm2(g-1)
            # so the PE never waits on the DVE hT cast; epilogue copies are
            # split across ScalarE/VectorE halves to stay off the critical
            # path; one output store per group on the sync queue.
            phs, pos = [None] * G, [None] * G

            def phclear(g):
                ph = phpool.tile([RANK, C0], f32, tag=f"ph{g % 2}", name=f"ph{g}")
                phs[g] = ph
                nc.vector.memset(ph[:, : cvec[g]], 0.0)

            def mm1(g):
                # Column-split into two 32-wide PE tiles: the LDWEIGHTS of
                # one tile overlaps the other tile's matmul (distinct
                # col_grps run concurrently), hiding the weight-load time.
                # The ph tile was DVE-memset to zero, so every matmul runs
                # start=False: stale has_written bits make it either
                # accumulate onto the zeros or overwrite its own elements;
                # both are correct, and the two 32-col tiles share a bank.
                xu_sb, w0, _, _ = slice_of(g)
                cg = cvec[g]
                ph = phs[g]
                for k in range(KC):
                    rhs = xu_sb[:, w0 + k * cg : w0 + (k + 1) * cg]
                    u0 = w0 + KC * cg + k * RANK
                    for t in range(2):
                        nc.tensor.matmul(
                            ph[32 * t : 32 * (t + 1), :cg],
                            lhsT=xu_sb[:, u0 + 32 * t : u0 + 32 * (t + 1)],
                            rhs=rhs,
                            start=False,
                            stop=(k == KC - 1 and t == 1),
                            skip_group_check=True,
                        )

            def hcast(g):
                # fp16 rounding of h^T; ones row is prefilled
                cg = cvec[g]
                nc.vector.tensor_copy(hTs[g % 3][0:RANK, :cg], phs[g][:, :cg])

            def mm2(g):
                # each 512-col half gets its own single-bank PSUM tile so
                # the epilogue of one half overlaps the matmul of the other
                _, _, vb_sb, gv = slice_of(g)
                cg = cvec[g]
                pot = [
                    popool.tile([C0, 512], f32, tag=f"po{j}_{g % 2}", name=f"po{j}_{g}")
                    for j in range(n2)
                ]
                pos[g] = pot
                for j in range(n2):
                    nc.tensor.matmul(
                        pot[j][:cg, :],
                        lhsT=hTs[g % 3][:, :cg],
                        rhs=vb_sb[:, gv, j * 512 : (j + 1) * 512],
                        start=True,
                        stop=True,
                    )

            o_pair = [None]

            def epilogue_store(g):
                # split the PSUM->SBUF cast across ScalarE/VectorE halves;
                # one store per group, issued as soon as its epilogue is
                # done, alternating the two HWDGE rings: finest drain
                # pipelining and the smallest possible final store.
                pot = pos[g]
                cg = cvec[g]
                if g % 2 == 0:
                    o_pair[0] = opool.tile([C0, 2, D_OUT], f16, tag="o", name=f"o{g}")
                o_sb = o_pair[0]
                half = D_OUT // 2
                nc.scalar.copy(o_sb[:cg, g % 2, 0:half], pot[0][:cg, :])
                nc.vector.tensor_copy(o_sb[:cg, g % 2, half:], pot[1][:cg, :])
                gp, sl = g // 2, g % 2
                eng = nc.sync if g % 2 == 0 else nc.scalar
                eng.dma_start(
                    out=o_d[gp, :cg, sl : sl + 1], in_=o_sb[:cg, sl : sl + 1]
                )

            # depth-2 pipeline: PE runs two groups of mm1 ahead of each
            # mm2, so the DVE hcast + its semaphore handshake are fully
            # hidden behind PE work.
            depth = min(2, G - 1) if G > 1 else 0
            for g in range(depth):
                phclear(g)
                mm1(g)
                hcast(g)
            for g in range(depth, G):
                phclear(g)
                mm1(g)
                mm2(g - depth)
                keep_warm(g)
                hcast(g)
                epilogue_store(g - depth)
            for g in range(G - depth, G):
                mm2(g)
                epilogue_store(g)

    nc.compile()
    return nc


def _route(pids: np.ndarray, n_experts: int):
    """Group token indices by expert, chunk to MAX_CHUNK, assign to cores.
    Each core's groups are sorted largest-first so a shared non-increasing
    per-group capacity vector wastes minimal padding."""
    order = np.argsort(pids, kind="stable")
    counts = np.bincount(pids, minlength=n_experts)
    per_core = n_experts // N_CORES
    core_groups = [[] for _ in range(N_CORES)]
    off = 0
    for p in range(n_experts):
        toks = order[off : off + counts[p]]
        off += counts[p]
        for s in range(0, len(toks), MAX_CHUNK):
            core_groups[p // per_core].append((p, toks[s : s + MAX_CHUNK]))
    for gs in core_groups:
        gs.sort(key=lambda g: -len(g[1]))
    return core_groups


def _capacity(core_groups):
    """G and the shared per-group capacity vector (non-increasing).

    Uniform capacity measured faster than a fitted per-group vector:
    smaller groups drop PE duty below the HAM un-throttle threshold and
    the matmul clock falls back to 1.2 GHz, costing more than the saved
    bytes/cycles.
    """
    G = max(len(gs) for gs in core_groups)
    maxlen = max((len(t) for gs in core_groups for _, t in gs), default=1)
    C = int(min(MAX_CHUNK, max(16, 4 * -(-maxlen // 4))))
    return G, (C,) * G


def _pack_core(groups, G, cvec, x8, U8, V8, bias8):
    """Build one core's in_map from its (pid, token) groups."""
    fws = [KC * c + KC * RANK for c in cvec]
    xoffs = [sum(fws[:g]) for g in range(G + 1)]
    xu = np.zeros((128, xoffs[G]), F8)
    vb = np.zeros((RANK + 1, G, D_OUT), F8)
    for gi, (p, toks) in enumerate(groups):
        cg = cvec[gi]
        w0 = xoffs[gi]
        blk = np.zeros((cg, D_IN), F8)
        blk[: len(toks)] = x8[toks]
        # [C, D] -> [d, t] -> [k, dp, t] -> [dp, k, t] -> flat [dp, k*t]
        xu[:, w0 : w0 + KC * cg] = (
            blk.T.reshape(KC, 128, cg).transpose(1, 0, 2).reshape(128, KC * cg)
        )
        xu[:, w0 + KC * cg : xoffs[gi + 1]] = (
            U8[p].reshape(KC, 128, RANK).transpose(1, 0, 2).reshape(128, KC * RANK)
        )
        vb[:RANK, gi] = V8[p]
        vb[RANK, gi] = bias8
    return {"xug": xu, "vbg": vb}


def _unpack(og_list, core_groups, N, bias_corr):
    # bias_corr = bias - fp8(bias): exact host-side correction of the fp8
    # bias row that rode through the matmul.
    out = np.zeros((N, D_OUT), np.float32)
    for c in range(N_CORES):
        og = np.asarray(og_list[c]).astype(np.float32)
        for gi, (p, toks) in enumerate(core_groups[c]):
            out[toks] = og[gi // 2, : len(toks), gi % 2] + bias_corr
    return out


def kernel(x, pids, U, V, bias):
    global LAST_EXEC_TIME_NS, LAST_RESULTS
    from concourse.bass_utils import run_bass_kernel_spmd

    x = np.asarray(x, dtype=np.float32)
    pids_np = np.asarray(pids).astype(np.int64)
    U = np.asarray(U, dtype=np.float32)
    V = np.asarray(V, dtype=np.float32)
    bias = np.asarray(bias, dtype=np.float32)

    N = x.shape[0]
    P = U.shape[0]

    x8 = x.astype(F8)
    U8 = U.astype(F8)
    V8 = V.astype(F8)
    bias8 = bias.astype(F8)
    bias_corr = bias - bias8.astype(np.float32)

    core_groups = _route(pids_np, P)
    G, cvec = _capacity(core_groups)

    in_maps = [
        _pack_core(core_groups[c], G, cvec, x8, U8, V8, bias8)
        for c in range(N_CORES)
    ]

    key = (G, cvec)
    if key not in _PROGRAM_CACHE:
        _PROGRAM_CACHE[key] = _build_program(G, cvec)
    nc = _PROGRAM_CACHE[key]

    trace = os.environ.get("KERNEL_TRACE", "0") == "1"
    res = run_bass_kernel_spmd(nc, in_maps, list(range(N_CORES)), trace=trace)
    LAST_EXEC_TIME_NS = res.exec_time_ns
    LAST_RESULTS = res

    return _unpack(
        [res.results[c]["og"] for c in range(N_CORES)], core_groups, N, bias_corr
    )


# revision 65
# speedup vs baseline: 1.0064x; 1.0064x over previous
"""Routed low-rank FFN (MoE-style) Trainium2 kernel.

out[n] = x[n] @ U[pids[n]] @ V[pids[n]] + bias

Strategy (expert-parallel over 8 NeuronCores), ~30us HW vs 50us baseline:
  - Host: stable-sort tokens by pid; expert p's tokens go to core p // 8.
    Each expert's token list is split into chunks of <= 128 tokens
    ("groups"); every core runs the same static program over G groups of
    capacity C (zero-padded), so the SPMD program is identical on all
    cores while the data differs.
  - Reduced-precision I/O (the kernel is DMA-bound at f32; the harness
    gate is rel_err < 2e-2, measured ~9.2e-3 for this mix):
      x, U, V, bias -> fp8 e4m3; h, out -> fp16; PSUM accumulates f32.
    The fp8 bias row's quantization error is corrected EXACTLY on the
    host by adding (bias - fp8(bias)) to the unpacked output.
  - Device, per group g (one expert's <=C tokens):
      h^T [64, C]    = sum_k U_chunk[k].T @ x_chunk[k]  (fp8, K=128,
        column-split into two 32-wide PE tiles so each LDWEIGHTS hides
        under the other tile's matmul; PSUM pre-zeroed by DVE so all
        matmuls run start=False and the tiles share one bank)
      out [C, 1024]  = [h^T; ones].T @ [V; bias]  (2 matmuls, fp16 x fp8,
        N=512 each into separate single-bank PSUM tiles)
  - Software pipeline at depth 2 (mm1(g) issues before mm2(g-2)) keeps
    the PE stream gapless; a 6-matmul warmup burst fills the dead head
    window and one small dummy matmul per group keeps PE duty high so
    the HAM clock un-throttle (1.2 -> 2.4 GHz) engages and sticks.
  - DMA: all loads on the sync HWDGE ring, interleaved in consumption
    order (x+U slice, V slice, ...; never SWDGE - its Q7 descriptor
    generation costs ~2us per DMA). Epilogue PSUM->SBUF casts split
    across ScalarE/VectorE halves; fp16 stores go out in pairs of
    groups alternating the two HWDGE rings, with the final pair split
    across both rings to shorten the last drain.
  - Host: inverse-permute rows back to original token order, cast f32.
"""

import os

import numpy as np
import ml_dtypes

N_CORES = 8
D_IN = 1024
RANK = 64
D_OUT = 1024
KC = 8  # number of 128-deep contraction chunks: D_IN // 128
MAX_CHUNK = 128  # max tokens per group (PE partition limit for matmul 2)

F8 = ml_dtypes.float8_e4m3
F16 = np.float16

# Set by kernel() after a traced run (KERNEL_TRACE=1): HW kernel span in ns.
LAST_EXEC_TIME_NS = None
LAST_RESULTS = None

_PROGRAM_CACHE = {}


def _slice_bounds(G, n_slices):
    """First slice small so compute starts early, rest even."""
    n_slices = max(1, min(n_slices, G))
    if n_slices == 1:
        return [0, G]
    rest = G - 1
    k = n_slices - 1
    bounds = [0, 1]
    for i in range(1, k + 1):
        bounds.append(1 + round(i * rest / k))
    return bounds


def _build_program(G: int, cvec: tuple):
    """Build the SPMD Bass/Tile program: G groups per core; group g has
    token capacity cvec[g] (non-increasing)."""
    import concourse.tile as tile
    from concourse import bacc, mybir

    nc = bacc.Bacc(
        "TRN2",
        target_bir_lowering=False,
        debug=False,
        enable_asserts=False,
        num_devices=N_CORES,
    )
    f32 = mybir.dt.float32
    f16 = mybir.dt.float16
    f8 = mybir.dt.float8e4

    C0 = cvec[0]
    # flat per-(partition, group) row: KC x-chunks of C_g, then KC u-chunks
    fws = [KC * c + KC * RANK for c in cvec]
    xoffs = [sum(fws[:g]) for g in range(G + 1)]
    FWT = xoffs[G]

    Gp = (G + 1) // 2  # output stores go out in pairs of groups
    xu_d = nc.dram_tensor("xug", [128, FWT], f8, kind="ExternalInput")
    vb_d = nc.dram_tensor("vbg", [RANK + 1, G, D_OUT], f8, kind="ExternalInput")
    o_d = nc.dram_tensor("og", [Gp, C0, 2, D_OUT], f16, kind="ExternalOutput")

    n2 = D_OUT // 512  # matmul-2 free-dim splits (one PSUM bank each)

    xbounds = _slice_bounds(G, 3)  # xu slices
    vbounds = _slice_bounds(G, 3)  # vb slices

    with tile.TileContext(nc) as tc:
        with (
            tc.tile_pool(name="xin", bufs=1) as xpool,
            tc.tile_pool(name="win", bufs=1) as wpool,
            tc.tile_pool(name="hbuf", bufs=1) as hpool,
            tc.tile_pool(name="obuf", bufs=3) as opool,
            tc.tile_pool(name="ph", bufs=1, space="PSUM") as phpool,
            tc.tile_pool(name="po", bufs=1, space="PSUM") as popool,
            tc.tile_pool(name="wm", bufs=1, space="PSUM") as wmpool,
        ):
            # f32 ones row; the [hT; ones] trick folds the bias matmul row.
            ones_sb = wpool.tile([1, C0], f32, tag="ones")
            nc.vector.memset(ones_sb[:], 1.0)

            # Three hT buffers with the fp16 ones row prefilled once.
            hTs = [
                hpool.tile([RANK + 1, C0], f16, tag=f"h{i}", name=f"hT{i}")
                for i in range(3)
            ]
            for i in range(3):
                nc.vector.tensor_copy(hTs[i][RANK : RANK + 1, :], ones_sb[:])

            # HAM warmup: dense N=512 matmuls sized to fit inside the dead
            # head window (preamble + first DMA latency), priming the PE
            # clock un-throttle at zero critical-path cost.
            wm_lhs = wpool.tile([RANK + 1, 32], f16, tag="wml")
            wm_rhs = wpool.tile([RANK + 1, 512], f16, tag="wmr")
            nc.vector.memset(wm_lhs[:], 0.0)
            nc.vector.memset(wm_rhs[:], 0.0)
            wm_ps = wmpool.tile([32, 512], f32, tag="wm")
            for _ in range(6):
                nc.tensor.matmul(
                    wm_ps[:], lhsT=wm_lhs[:], rhs=wm_rhs[:], start=True, stop=True
                )

            def keep_warm(g):
                # one modest dummy matmul per group keeps PE activity dense
                # enough for the HAM clock un-throttle to engage and stick.
                nc.tensor.matmul(
                    wm_ps[:, 0:256],
                    lhsT=wm_lhs[:],
                    rhs=wm_rhs[:, 0:256],
                    start=True,
                    stop=True,
                )

            # All loads on ONE HWDGE ring (sync), interleaved in the order
            # compute consumes them: the ring drains FIFO at the full
            # engine-aggregate rate, so slice k never waits behind bytes
            # that are only needed later. Stores go on the other ring.
            xu_parts, vb_parts = [], []
            for s in range(max(len(xbounds), len(vbounds)) - 1):
                if s < len(xbounds) - 1:
                    g0, g1 = xbounds[s], xbounds[s + 1]
                    w0, w1 = xoffs[g0], xoffs[g1]
                    xu_sb = xpool.tile([128, w1 - w0], f8, tag=f"xu{s}")
                    nc.sync.dma_start(out=xu_sb[:], in_=xu_d[:, w0:w1])
                    xu_parts.append(xu_sb)
                if s < len(vbounds) - 1:
                    g0, g1 = vbounds[s], vbounds[s + 1]
                    vb_sb = wpool.tile([RANK + 1, g1 - g0, D_OUT], f8, tag=f"vb{s}")
                    nc.sync.dma_start(out=vb_sb[:], in_=vb_d[:, g0:g1])
                    vb_parts.append(vb_sb)

            def slice_of(g):
                sx = next(i for i in range(len(xbounds) - 1) if xbounds[i + 1] > g)
                sv = next(i for i in range(len(vbounds) - 1) if vbounds[i + 1] > g)
                return (
                    xu_parts[sx],
                    xoffs[g] - xoffs[xbounds[sx]],
                    vb_parts[sv],
                    g - vbounds[sv],
                )

            # Software pipeline over groups: issue mm1(g) before mm2(g-1)
            # so the PE never waits on the DVE hT cast; epilogue copies are
            # split across ScalarE/VectorE halves to stay off the critical
            # path; one output store per group on the sync queue.
            phs, pos = [None] * G, [None] * G

            def phclear(g):
                ph = phpool.tile([RANK, C0], f32, tag=f"ph{g % 2}", name=f"ph{g}")
                phs[g] = ph

            def mm1(g):
                xu_sb, w0, _, _ = slice_of(g)
                cg = cvec[g]
                ph = phs[g]
                for k in range(KC):
                    nc.tensor.matmul(
                        ph[:, :cg],
                        lhsT=xu_sb[:, w0 + KC * cg + k * RANK : w0 + KC * cg + (k + 1) * RANK],
                        rhs=xu_sb[:, w0 + k * cg : w0 + (k + 1) * cg],
                        start=(k == 0),
                        stop=(k == KC - 1),
                    )

            def hcast(g):
                # fp16 rounding of h^T; ones row is prefilled
                cg = cvec[g]
                nc.vector.tensor_copy(hTs[g % 3][0:RANK, :cg], phs[g][:, :cg])

            def mm2(g):
                # each 512-col half gets its own single-bank PSUM tile so
                # the epilogue of one half overlaps the matmul of the other
                _, _, vb_sb, gv = slice_of(g)
                cg = cvec[g]
                pot = [
                    popool.tile([C0, 512], f32, tag=f"po{j}_{g % 2}", name=f"po{j}_{g}")
                    for j in range(n2)
                ]
                pos[g] = pot
                for j in range(n2):
                    nc.tensor.matmul(
                        pot[j][:cg, :],
                        lhsT=hTs[g % 3][:, :cg],
                        rhs=vb_sb[:, gv, j * 512 : (j + 1) * 512],
                        start=True,
                        stop=True,
                    )

            o_pair = [None]

            def epilogue_store(g):
                # split the PSUM->SBUF cast across ScalarE/VectorE halves;
                # one store per group, issued as soon as its epilogue is
                # done, alternating the two HWDGE rings: finest drain
                # pipelining and the smallest possible final store.
                pot = pos[g]
                cg = cvec[g]
                if g % 2 == 0:
                    o_pair[0] = opool.tile([C0, 2, D_OUT], f16, tag="o", name=f"o{g}")
                o_sb = o_pair[0]
                half = D_OUT // 2
                nc.scalar.copy(o_sb[:cg, g % 2, 0:half], pot[0][:cg, :])
                nc.vector.tensor_copy(o_sb[:cg, g % 2, half:], pot[1][:cg, :])
                gp, sl = g // 2, g % 2
                eng = nc.sync if g % 2 == 0 else nc.scalar
                eng.dma_start(
                    out=o_d[gp, :cg, sl : sl + 1], in_=o_sb[:cg, sl : sl + 1]
                )

            # depth-2 pipeline: PE runs two groups of mm1 ahead of each
            # mm2, so the DVE hcast + its semaphore handshake are fully
            # hidden behind PE work.
            depth = min(2, G - 1) if G > 1 else 0
            for g in range(depth):
                phclear(g)
                mm1(g)
                hcast(g)
            for g in range(depth, G):
                phclear(g)
                mm1(g)
                mm2(g - depth)
                keep_warm(g)
                hcast(g)
                epilogue_store(g - depth)
            for g in range(G - depth, G):
                mm2(g)
                epilogue_store(g)

    nc.compile()
    return nc


def _route(pids: np.ndarray, n_experts: int):
    """Group token indices by expert, chunk to MAX_CHUNK, assign to cores.
    Each core's groups are sorted largest-first so a shared non-increasing
    per-group capacity vector wastes minimal padding."""
    order = np.argsort(pids, kind="stable")
    counts = np.bincount(pids, minlength=n_experts)
    per_core = n_experts // N_CORES
    core_groups = [[] for _ in range(N_CORES)]
    off = 0
    for p in range(n_experts):
        toks = order[off : off + counts[p]]
        off += counts[p]
        for s in range(0, len(toks), MAX_CHUNK):
            core_groups[p // per_core].append((p, toks[s : s + MAX_CHUNK]))
    for gs in core_groups:
        gs.sort(key=lambda g: -len(g[1]))
    return core_groups


def _capacity(core_groups):
    """G and the shared per-group capacity vector (non-increasing).

    Uniform capacity measured faster than a fitted per-group vector:
    smaller groups drop PE duty below the HAM un-throttle threshold and
    the matmul clock falls back to 1.2 GHz, costing more than the saved
    bytes/cycles.
    """
    G = max(len(gs) for gs in core_groups)
    maxlen = max((len(t) for gs in core_groups for _, t in gs), default=1)
    C = int(min(MAX_CHUNK, max(16, 4 * -(-maxlen // 4))))
    return G, (C,) * G


def _pack_core(groups, G, cvec, x8, U8, V8, bias8):
    """Build one core's in_map from its (pid, token) groups."""
    fws = [KC * c + KC * RANK for c in cvec]
    xoffs = [sum(fws[:g]) for g in range(G + 1)]
    xu = np.zeros((128, xoffs[G]), F8)
    vb = np.zeros((RANK + 1, G, D_OUT), F8)
    for gi, (p, toks) in enumerate(groups):
        cg = cvec[gi]
        w0 = xoffs[gi]
        blk = np.zeros((cg, D_IN), F8)
        blk[: len(toks)] = x8[toks]
        # [C, D] -> [d, t] -> [k, dp, t] -> [dp, k, t] -> flat [dp, k*t]
        xu[:, w0 : w0 + KC * cg] = (
            blk.T.reshape(KC, 128, cg).transpose(1, 0, 2).reshape(128, KC * cg)
        )
        xu[:, w0 + KC * cg : xoffs[gi + 1]] = (
            U8[p].reshape(KC, 128, RANK).transpose(1, 0, 2).reshape(128, KC * RANK)
        )
        vb[:RANK, gi] = V8[p]
        vb[RANK, gi] = bias8
    return {"xug": xu, "vbg": vb}


def _unpack(og_list, core_groups, N, bias_corr):
    # bias_corr = bias - fp8(bias): exact host-side correction of the fp8
    # bias row that rode through the matmul.
    out = np.zeros((N, D_OUT), np.float32)
    for c in range(N_CORES):
        og = np.asarray(og_list[c]).astype(np.float32)
        for gi, (p, toks) in enumerate(core_groups[c]):
            out[toks] = og[gi // 2, : len(toks), gi % 2] + bias_corr
    return out


def kernel(x, pids, U, V, bias):
    global LAST_EXEC_TIME_NS, LAST_RESULTS
    from concourse.bass_utils import run_bass_kernel_spmd

    x = np.asarray(x, dtype=np.float32)
    pids_np = np.asarray(pids).astype(np.int64)
    U = np.asarray(U, dtype=np.float32)
    V = np.asarray(V, dtype=np.float32)
    bias = np.asarray(bias, dtype=np.float32)

    N = x.shape[0]
    P = U.shape[0]

    x8 = x.astype(F8)
    U8 = U.astype(F8)
    V8 = V.astype(F8)
    bias8 = bias.astype(F8)
    bias_corr = bias - bias8.astype(np.float32)

    core_groups = _route(pids_np, P)
    G, cvec = _capacity(core_groups)

    in_maps = [
        _pack_core(core_groups[c], G, cvec, x8, U8, V8, bias8)
        for c in range(N_CORES)
    ]

    key = (G, cvec)
    if key not in _PROGRAM_CACHE:
        _PROGRAM_CACHE[key] = _build_program(G, cvec)
    nc = _PROGRAM_CACHE[key]

    trace = os.environ.get("KERNEL_TRACE", "0") == "1"
    res = run_bass_kernel_spmd(nc, in_maps, list(range(N_CORES)), trace=trace)
    LAST_EXEC_TIME_NS = res.exec_time_ns
    LAST_RESULTS = res

    return _unpack(
        [res.results[c]["og"] for c in range(N_CORES)], core_groups, N, bias_corr
    )


# revision 66
# speedup vs baseline: 1.0949x; 1.0880x over previous
"""Routed low-rank FFN (MoE-style) Trainium2 kernel.

out[n] = x[n] @ U[pids[n]] @ V[pids[n]] + bias

Strategy (expert-parallel over 8 NeuronCores), ~30us HW vs 50us baseline:
  - Host: stable-sort tokens by pid; expert p's tokens go to core p // 8.
    Each expert's token list is split into chunks of <= 128 tokens
    ("groups"); every core runs the same static program over G groups of
    capacity C (zero-padded), so the SPMD program is identical on all
    cores while the data differs.
  - Reduced-precision I/O (the kernel is DMA-bound at f32; the harness
    gate is rel_err < 2e-2, measured ~9.2e-3 for this mix):
      x, U, V, bias -> fp8 e4m3; h, out -> fp16; PSUM accumulates f32.
    The fp8 bias row's quantization error is corrected EXACTLY on the
    host by adding (bias - fp8(bias)) to the unpacked output.
  - Device, per group g (one expert's <=C tokens):
      h^T [64, C]    = sum_k U_chunk[k].T @ x_chunk[k]  (8 fp8 matmuls,
        K=128, accumulated in one PSUM bank)
      out [C, 1024]  = [h^T; ones].T @ [V; bias]  (2 matmuls, fp16 x fp8,
        N=512 each into separate single-bank PSUM tiles)
  - Software pipeline at depth 2 (mm1(g) issues before mm2(g-2)) keeps
    the PE stream gapless; a 6-matmul warmup burst fills the dead head
    window and one small dummy matmul per group keeps PE duty high so
    the HAM clock un-throttle (1.2 -> 2.4 GHz) can engage and stick
    (it is flaky on this part; the schedule is also fast fully cold).
  - DMA: all loads on the sync HWDGE ring, interleaved in consumption
    order (x+U slice, V slice, ...; never SWDGE - its Q7 descriptor
    generation costs ~2us per DMA). Epilogue PSUM->SBUF casts split
    across ScalarE/VectorE halves; each group's fp16 output is stored
    as soon as its epilogue finishes, alternating the two HWDGE rings,
    so the final drain is one small store.
  - Host: inverse-permute rows back to original token order, cast f32.
"""

import os

import numpy as np
import ml_dtypes

N_CORES = 8
D_IN = 1024
RANK = 64
D_OUT = 1024
KC = 8  # number of 128-deep contraction chunks: D_IN // 128
MAX_CHUNK = 128  # max tokens per group (PE partition limit for matmul 2)

F8 = ml_dtypes.float8_e4m3
F16 = np.float16

# Set by kernel() after a traced run (KERNEL_TRACE=1): HW kernel span in ns.
LAST_EXEC_TIME_NS = None
LAST_RESULTS = None

_PROGRAM_CACHE = {}


def _slice_bounds(G, n_slices):
    """First slice small so compute starts early, rest even."""
    n_slices = max(1, min(n_slices, G))
    if n_slices == 1:
        return [0, G]
    rest = G - 1
    k = n_slices - 1
    bounds = [0, 1]
    for i in range(1, k + 1):
        bounds.append(1 + round(i * rest / k))
    return bounds


def _build_program(G: int, cvec: tuple):
    """Build the SPMD Bass/Tile program: G groups per core; group g has
    token capacity cvec[g] (non-increasing)."""
    import concourse.tile as tile
    from concourse import bacc, mybir

    nc = bacc.Bacc(
        "TRN2",
        target_bir_lowering=False,
        debug=False,
        enable_asserts=False,
        num_devices=N_CORES,
    )
    f32 = mybir.dt.float32
    f16 = mybir.dt.float16
    f8 = mybir.dt.float8e4

    C0 = cvec[0]
    # flat per-(partition, group) row: KC x-chunks of C_g, then KC u-chunks
    fws = [KC * c + KC * RANK for c in cvec]
    xoffs = [sum(fws[:g]) for g in range(G + 1)]
    FWT = xoffs[G]

    Gp = (G + 1) // 2  # output stores go out in pairs of groups
    xu_d = nc.dram_tensor("xug", [128, FWT], f8, kind="ExternalInput")
    vb_d = nc.dram_tensor("vbg", [RANK + 1, G, D_OUT], f8, kind="ExternalInput")
    o_d = nc.dram_tensor("og", [Gp, C0, 2, D_OUT], f16, kind="ExternalOutput")

    n2 = D_OUT // 512  # matmul-2 free-dim splits (one PSUM bank each)

    xbounds = _slice_bounds(G, 3)  # xu slices
    vbounds = _slice_bounds(G, 3)  # vb slices

    with tile.TileContext(nc) as tc:
        with (
            tc.tile_pool(name="xin", bufs=1) as xpool,
            tc.tile_pool(name="win", bufs=1) as wpool,
            tc.tile_pool(name="hbuf", bufs=1) as hpool,
            tc.tile_pool(name="obuf", bufs=3) as opool,
            tc.tile_pool(name="ph", bufs=1, space="PSUM") as phpool,
            tc.tile_pool(name="po", bufs=1, space="PSUM") as popool,
            tc.tile_pool(name="wm", bufs=1, space="PSUM") as wmpool,
        ):
            # f32 ones row; the [hT; ones] trick folds the bias matmul row.
            ones_sb = wpool.tile([1, C0], f32, tag="ones")
            nc.vector.memset(ones_sb[:], 1.0)

            # Three hT buffers with the fp16 ones row prefilled once.
            hTs = [
                hpool.tile([RANK + 1, C0], f16, tag=f"h{i}", name=f"hT{i}")
                for i in range(3)
            ]
            for i in range(3):
                nc.vector.tensor_copy(hTs[i][RANK : RANK + 1, :], ones_sb[:])

            # HAM warmup: dense N=512 matmuls sized to fit inside the dead
            # head window (preamble + first DMA latency), priming the PE
            # clock un-throttle at zero critical-path cost.
            wm_lhs = wpool.tile([RANK + 1, 32], f16, tag="wml")
            wm_rhs = wpool.tile([RANK + 1, 512], f16, tag="wmr")
            nc.vector.memset(wm_lhs[:], 0.0)
            nc.vector.memset(wm_rhs[:], 0.0)
            wm_ps = wmpool.tile([32, 512], f32, tag="wm")
            for _ in range(6):
                nc.tensor.matmul(
                    wm_ps[:], lhsT=wm_lhs[:], rhs=wm_rhs[:], start=True, stop=True
                )

            def keep_warm(g):
                # one modest dummy matmul per group keeps PE activity dense
                # enough for the HAM clock un-throttle to engage and stick.
                nc.tensor.matmul(
                    wm_ps[:, 0:256],
                    lhsT=wm_lhs[:],
                    rhs=wm_rhs[:, 0:256],
                    start=True,
                    stop=True,
                )

            # All loads on ONE HWDGE ring (sync), interleaved in the order
            # compute consumes them: the ring drains FIFO at the full
            # engine-aggregate rate, so slice k never waits behind bytes
            # that are only needed later. Stores go on the other ring.
            xu_parts, vb_parts = [], []
            for s in range(max(len(xbounds), len(vbounds)) - 1):
                if s < len(xbounds) - 1:
                    g0, g1 = xbounds[s], xbounds[s + 1]
                    w0, w1 = xoffs[g0], xoffs[g1]
                    xu_sb = xpool.tile([128, w1 - w0], f8, tag=f"xu{s}")
                    nc.sync.dma_start(out=xu_sb[:], in_=xu_d[:, w0:w1])
                    xu_parts.append(xu_sb)
                if s < len(vbounds) - 1:
                    g0, g1 = vbounds[s], vbounds[s + 1]
                    vb_sb = wpool.tile([RANK + 1, g1 - g0, D_OUT], f8, tag=f"vb{s}")
                    nc.sync.dma_start(out=vb_sb[:], in_=vb_d[:, g0:g1])
                    vb_parts.append(vb_sb)

            def slice_of(g):
                sx = next(i for i in range(len(xbounds) - 1) if xbounds[i + 1] > g)
                sv = next(i for i in range(len(vbounds) - 1) if vbounds[i + 1] > g)
                return (
                    xu_parts[sx],
                    xoffs[g] - xoffs[xbounds[sx]],
                    vb_parts[sv],
                    g - vbounds[sv],
                )

            # Software pipeline over groups: issue mm1(g) before mm2(g-1)
            # so the PE never waits on the DVE hT cast; epilogue copies are
            # split across ScalarE/VectorE halves to stay off the critical
            # path; one output store per group on the sync queue.
            phs, pos = [None] * G, [None] * G

            def phclear(g):
                ph = phpool.tile([RANK, C0], f32, tag=f"ph{g % 2}", name=f"ph{g}")
                phs[g] = ph

            def mm1(g):
                xu_sb, w0, _, _ = slice_of(g)
                cg = cvec[g]
                ph = phs[g]
                for k in range(KC):
                    nc.tensor.matmul(
                        ph[:, :cg],
                        lhsT=xu_sb[:, w0 + KC * cg + k * RANK : w0 + KC * cg + (k + 1) * RANK],
                        rhs=xu_sb[:, w0 + k * cg : w0 + (k + 1) * cg],
                        start=(k == 0),
                        stop=(k == KC - 1),
                    )

            def hcast(g):
                # fp16 rounding of h^T; ones row is prefilled
                cg = cvec[g]
                nc.vector.tensor_copy(hTs[g % 3][0:RANK, :cg], phs[g][:, :cg])

            def mm2(g):
                # each 512-col half gets its own single-bank PSUM tile so
                # the epilogue of one half overlaps the matmul of the other
                _, _, vb_sb, gv = slice_of(g)
                cg = cvec[g]
                pot = [
                    popool.tile([C0, 512], f32, tag=f"po{j}_{g % 2}", name=f"po{j}_{g}")
                    for j in range(n2)
                ]
                pos[g] = pot
                for j in range(n2):
                    nc.tensor.matmul(
                        pot[j][:cg, :],
                        lhsT=hTs[g % 3][:, :cg],
                        rhs=vb_sb[:, gv, j * 512 : (j + 1) * 512],
                        start=True,
                        stop=True,
                    )

            o_pair = [None]

            def epilogue_store(g):
                # split the PSUM->SBUF cast across ScalarE/VectorE halves;
                # one store per group, issued as soon as its epilogue is
                # done, alternating the two HWDGE rings: finest drain
                # pipelining and the smallest possible final store.
                pot = pos[g]
                cg = cvec[g]
                if g % 2 == 0:
                    o_pair[0] = opool.tile([C0, 2, D_OUT], f16, tag="o", name=f"o{g}")
                o_sb = o_pair[0]
                half = D_OUT // 2
                nc.scalar.copy(o_sb[:cg, g % 2, 0:half], pot[0][:cg, :])
                nc.vector.tensor_copy(o_sb[:cg, g % 2, half:], pot[1][:cg, :])
                gp, sl = g // 2, g % 2
                eng = nc.sync if g % 2 == 0 else nc.scalar
                eng.dma_start(
                    out=o_d[gp, :cg, sl : sl + 1], in_=o_sb[:cg, sl : sl + 1]
                )

            # depth-2 pipeline: PE runs two groups of mm1 ahead of each
            # mm2, so the DVE hcast + its semaphore handshake are fully
            # hidden behind PE work.
            depth = min(2, G - 1) if G > 1 else 0
            for g in range(depth):
                phclear(g)
                mm1(g)
                hcast(g)
            for g in range(depth, G):
                phclear(g)
                mm1(g)
                mm2(g - depth)
                keep_warm(g)
                hcast(g)
                epilogue_store(g - depth)
            for g in range(G - depth, G):
                mm2(g)
                epilogue_store(g)

    nc.compile()
    return nc


def _route(pids: np.ndarray, n_experts: int):
    """Group token indices by expert, chunk to MAX_CHUNK, assign to cores.
    Each core's groups are sorted largest-first so a shared non-increasing
    per-group capacity vector wastes minimal padding."""
    order = np.argsort(pids, kind="stable")
    counts = np.bincount(pids, minlength=n_experts)
    per_core = n_experts // N_CORES
    core_groups = [[] for _ in range(N_CORES)]
    off = 0
    for p in range(n_experts):
        toks = order[off : off + counts[p]]
        off += counts[p]
        for s in range(0, len(toks), MAX_CHUNK):
            core_groups[p // per_core].append((p, toks[s : s + MAX_CHUNK]))
    for gs in core_groups:
        gs.sort(key=lambda g: -len(g[1]))
    return core_groups


def _capacity(core_groups):
    """G and the shared per-group capacity vector (non-increasing).

    Uniform capacity measured faster than a fitted per-group vector:
    smaller groups drop PE duty below the HAM un-throttle threshold and
    the matmul clock falls back to 1.2 GHz, costing more than the saved
    bytes/cycles.
    """
    G = max(len(gs) for gs in core_groups)
    maxlen = max((len(t) for gs in core_groups for _, t in gs), default=1)
    C = int(min(MAX_CHUNK, max(16, 4 * -(-maxlen // 4))))
    return G, (C,) * G


def _pack_core(groups, G, cvec, x8, U8, V8, bias8):
    """Build one core's in_map from its (pid, token) groups."""
    fws = [KC * c + KC * RANK for c in cvec]
    xoffs = [sum(fws[:g]) for g in range(G + 1)]
    xu = np.zeros((128, xoffs[G]), F8)
    vb = np.zeros((RANK + 1, G, D_OUT), F8)
    for gi, (p, toks) in enumerate(groups):
        cg = cvec[gi]
        w0 = xoffs[gi]
        blk = np.zeros((cg, D_IN), F8)
        blk[: len(toks)] = x8[toks]
        # [C, D] -> [d, t] -> [k, dp, t] -> [dp, k, t] -> flat [dp, k*t]
        xu[:, w0 : w0 + KC * cg] = (
            blk.T.reshape(KC, 128, cg).transpose(1, 0, 2).reshape(128, KC * cg)
        )
        xu[:, w0 + KC * cg : xoffs[gi + 1]] = (
            U8[p].reshape(KC, 128, RANK).transpose(1, 0, 2).reshape(128, KC * RANK)
        )
        vb[:RANK, gi] = V8[p]
        vb[RANK, gi] = bias8
    return {"xug": xu, "vbg": vb}


def _unpack(og_list, core_groups, N, bias_corr):
    # bias_corr = bias - fp8(bias): exact host-side correction of the fp8
    # bias row that rode through the matmul.
    out = np.zeros((N, D_OUT), np.float32)
    for c in range(N_CORES):
        og = np.asarray(og_list[c]).astype(np.float32)
        for gi, (p, toks) in enumerate(core_groups[c]):
            out[toks] = og[gi // 2, : len(toks), gi % 2] + bias_corr
    return out


def kernel(x, pids, U, V, bias):
    global LAST_EXEC_TIME_NS, LAST_RESULTS
    from concourse.bass_utils import run_bass_kernel_spmd

    x = np.asarray(x, dtype=np.float32)
    pids_np = np.asarray(pids).astype(np.int64)
    U = np.asarray(U, dtype=np.float32)
    V = np.asarray(V, dtype=np.float32)
    bias = np.asarray(bias, dtype=np.float32)

    N = x.shape[0]
    P = U.shape[0]

    x8 = x.astype(F8)
    U8 = U.astype(F8)
    V8 = V.astype(F8)
    bias8 = bias.astype(F8)
    bias_corr = bias - bias8.astype(np.float32)

    core_groups = _route(pids_np, P)
    G, cvec = _capacity(core_groups)

    in_maps = [
        _pack_core(core_groups[c], G, cvec, x8, U8, V8, bias8)
        for c in range(N_CORES)
    ]

    key = (G, cvec)
    if key not in _PROGRAM_CACHE:
        _PROGRAM_CACHE[key] = _build_program(G, cvec)
    nc = _PROGRAM_CACHE[key]

    trace = os.environ.get("KERNEL_TRACE", "0") == "1"
    res = run_bass_kernel_spmd(nc, in_maps, list(range(N_CORES)), trace=trace)
    LAST_EXEC_TIME_NS = res.exec_time_ns
    LAST_RESULTS = res

    return _unpack(
        [res.results[c]["og"] for c in range(N_CORES)], core_groups, N, bias_corr
    )
